# revision 4
# baseline (speedup 1.0000x reference)
"""Trainium2 Bass kernel for nn_AdversarialHead (scatter_memory).

Computes, for F*A = 131072 rows:
  one_hot  = multi-hot(actions + starts)                      [rows, 21]
  h        = leaky_relu(fc1_w @ [cur; one_hot] + fc1_b, 0.1)  [rows, 140]
  pred     = fc2_w @ h + fc2_b                                [rows, 128]
  logits_i = head_wi @ [cur; nxt] + head_bi                   [rows, 5/7/9]

Strategy (pure data parallel over 8 cores, 16384 rows each):
- Host pre-transposes cur/nxt to [feat, rows] so every device matmul is a
  plain orientation-B matmul (features on partitions, rows on the free dim).
- The one-hot scatter becomes: per-partition is_equal against an iota
  constant (building one-hotT tiles on chip), then a small-K matmul against
  host-prebuilt embedding tables (fc1_w columns re-indexed by action value).
  All biases are folded into the row-15 "ones" row of those tables.
- leaky_relu is decomposed as 0.1*x + 0.9*relu(x); the linear 0.1*x path is
  folded into the weights (Wfuse = 0.1*fc2_w@fc1_w) so the device only needs
  a single Relu per PSUM tile; the relu path uses 0.9-scaled fc2 weights.
- f32 mode uses float32r matmuls (full-rate PE); bf16 mode halves HBM bytes.

Outputs are produced transposed ([feat, rows]) and un-transposed on host.
"""
import os

import numpy as np
import ml_dtypes

import concourse.bass as bass
import concourse.tile as tile
from concourse import bacc, mybir
from concourse.bass_utils import run_bass_kernel_spmd

# ---------------------------------------------------------------- constants
F_FRAMES, A_AGENTS, FEAT = 4096, 32, 128
NVEC = [5, 7, 9]
STARTS = [0, 5, 12]
ACT_TOTAL = 21            # sum(NVEC)
HID = 140
NCORES = 8
ROWS = F_FRAMES * A_AGENTS          # 131072
RPC = ROWS // NCORES                # 16384 rows per core
TILE_N = 1024                       # rows per megatile
NT = RPC // TILE_N                  # 16 megatiles per core
HID2 = HID - FEAT                   # 12 hidden dims beyond partition 128

# psum2 layout: rows 0:21 = logits, rows 21:32 = zero pad, rows 32:44 = h2
# (compute-engine PSUM reads must start at a 32-aligned partition)
NL = 44
H2_BASE = 32

# Variant: "f32" (float32 I/O, float32r matmuls) or "bf16"
VARIANT = os.environ.get("KERNEL_DTYPE", "f32")

_prog_cache = {}


def _np_dt(variant):
    return np.float32 if variant == "f32" else ml_dtypes.bfloat16


def _prep_weights(fc1_w, fc1_b, fc2_w, fc2_b,
                  head_w0, head_b0, head_w1, head_b1, head_w2, head_b2):
    """Build all stationary (lhsT) operands in float32. Shapes noted as [K, M]."""
    f32 = np.float32
    fc1_w = np.asarray(fc1_w, f32); fc1_b = np.asarray(fc1_b, f32)
    fc2_w = np.asarray(fc2_w, f32); fc2_b = np.asarray(fc2_b, f32)
    wcat = np.concatenate([np.asarray(head_w0, f32), np.asarray(head_w1, f32),
                           np.asarray(head_w2, f32)], axis=0)          # [21, 256]
    bcat = np.concatenate([np.asarray(head_b0, f32), np.asarray(head_b1, f32),
                           np.asarray(head_b2, f32)], axis=0)          # [21]

    wfuse = 0.1 * (fc2_w @ fc1_w)                                      # [128, 149]

    # embedding row index map: p = t*5 + v  ->  fc1 input column 128+STARTS[t]+v
    emb_cols = [FEAT + STARTS[t] + v for t in range(3) for v in range(5)]

    w = {}
    w["wa"] = fc1_w[:FEAT, :FEAT].T.copy()              # [128, 128] h1 from cur
    wb = np.zeros((FEAT, NL), f32)                      # [128, 44] logits+h2 from cur
    wb[:, :ACT_TOTAL] = wcat[:, :FEAT].T
    wb[:, H2_BASE:] = fc1_w[FEAT:, :FEAT].T
    w["wb"] = wb
    wc = np.zeros((16, FEAT), f32)                      # [16, 128] h1 from one-hot
    wc[:15, :] = fc1_w[:FEAT, emb_cols].T
    wc[15, :] = fc1_b[:FEAT]
    w["wc"] = wc
    wd = np.zeros((16, NL), f32)                        # [16, 44] h2+head biases
    wd[:15, H2_BASE:] = fc1_w[FEAT:, emb_cols].T
    wd[15, H2_BASE:] = fc1_b[FEAT:]
    wd[15, :ACT_TOTAL] = bcat
    w["wd"] = wd
    we = np.zeros((FEAT, NL), f32)                      # [128, 44] logits from nxt
    we[:, :ACT_TOTAL] = wcat[:, FEAT:].T
    w["we"] = we
    w["wi"] = wfuse[:, :FEAT].T.copy()                  # [128, 128] 0.1-path from cur
    wj = np.zeros((16, FEAT), f32)                      # [16, 128] 0.1-path from one-hot
    wj[:15, :] = wfuse[:, emb_cols].T
    wj[15, :] = 0.1 * (fc2_w @ fc1_b) + fc2_b
    w["wj"] = wj
    w["wf2"] = (0.9 * fc2_w[:, :FEAT]).T.copy()         # [128, 128] relu path h1
    w["wg2"] = (0.9 * fc2_w[:, FEAT:]).T.copy()         # [12, 128] relu path h2
    return w


def _build_program(variant):
    """Build the SPMD Bass program (identical across cores)."""
    if variant == "f32":
        dt_dat = mybir.dt.float32r     # matmul operands, stored as f32
        dt_out = mybir.dt.float32
    else:
        dt_dat = mybir.dt.bfloat16
        dt_out = mybir.dt.bfloat16
    dt_act = mybir.dt.bfloat16         # action values (0..4, exact in bf16)
    f32 = mybir.dt.float32

    AF = mybir.ActivationFunctionType
    OP = mybir.AluOpType

    nc = bacc.Bacc(None, target_bir_lowering=False, debug=False)

    def din(name, shape, dt):
        return nc.dram_tensor(name, list(shape), dt, kind="ExternalInput").ap()

    def dout(name, shape, dt):
        return nc.dram_tensor(name, list(shape), dt, kind="ExternalOutput").ap()

    curT = din("curT", (FEAT, RPC), dt_dat)
    nxtT = din("nxtT", (FEAT, RPC), dt_dat)
    act80 = din("act80", (80, RPC), dt_act)
    w_shapes = {"wa": (FEAT, FEAT), "wb": (FEAT, NL), "wc": (16, FEAT),
                "wd": (16, NL), "we": (FEAT, NL), "wi": (FEAT, FEAT),
                "wj": (16, FEAT), "wf2": (FEAT, FEAT), "wg2": (HID2, FEAT)}
    # wd / wj / wg2 are loaded at 32-aligned partition offsets so their
    # small-K matmuls occupy distinct PE row groups
    w_dram = {k: din(k, s, dt_dat) for k, s in w_shapes.items()}
    iota80 = din("iota80", (80, 1), f32)
    predT = dout("predT", (FEAT, RPC), dt_out)
    logitsT = dout("logitsT", (ACT_TOTAL, RPC), f32)

    with tile.TileContext(nc) as tc:
        with (
            tc.tile_pool(name="consts", bufs=1) as cp,
            tc.tile_pool(name="io", bufs=3) as iop,
            tc.tile_pool(name="mid", bufs=3) as mp,
            tc.tile_pool(name="pp1", bufs=1, space="PSUM") as pp1,
            tc.tile_pool(name="pp2", bufs=2, space="PSUM") as pp2,
            tc.tile_pool(name="pp3", bufs=1, space="PSUM") as pp3,
        ):
            # ---- load constants once ----
            cw = {}
            for k in ("wa", "wb", "wc", "we", "wi", "wf2"):
                t = cp.tile(w_shapes[k], dt_dat, name=f"c_{k}")
                nc.sync.dma_start(t[:], w_dram[k])
                cw[k] = t
            t = cp.tile([NL, FEAT], dt_dat, name="c_wg2")
            nc.sync.dma_start(t[H2_BASE:NL, :], w_dram["wg2"])
            cw["wg2"] = t
            # wd lives at partitions 32..47, wj at 64..79 so their small-K
            # matmuls land in distinct PE row-groups and run concurrently.
            t = cp.tile([48, NL], dt_dat, name="c_wd")
            nc.sync.dma_start(t[32:48, :], w_dram["wd"])
            cw["wd"] = t
            t = cp.tile([80, FEAT], dt_dat, name="c_wj")
            nc.sync.dma_start(t[64:80, :], w_dram["wj"])
            cw["wj"] = t
            iota_t = cp.tile([80, 1], f32, name="c_iota")
            nc.sync.dma_start(iota_t[:], iota80)

            for it in range(NT):
                sl = slice(it * TILE_N, (it + 1) * TILE_N)
                cur_t = iop.tile([FEAT, TILE_N], dt_dat, tag="cur")
                nc.sync.dma_start(cur_t[:], curT[:, sl])
                nxt_t = iop.tile([FEAT, TILE_N], dt_dat, tag="nxt")
                nc.sync.dma_start(nxt_t[:], nxtT[:, sl])
                act_t = iop.tile([80, TILE_N], dt_act, tag="act")
                nc.sync.dma_start(act_t[:], act80[:, sl])

                # one-hotT tiles (replicated in three row-groups; row15 of each
                # group is all-ones because the action row is 0 and iota is 0)
                oh = mp.tile([80, TILE_N], dt_dat, tag="oh")
                nc.gpsimd.tensor_scalar(oh[:], act_t[:], iota_t[:], None,
                                        OP.is_equal)

                p1 = pp1.tile([FEAT, TILE_N], f32, tag="p1")   # h1 pre-act
                p2 = pp2.tile([NL, TILE_N], f32, tag="p2")     # h2 + logits
                p3 = pp3.tile([FEAT, TILE_N], f32, tag="p3")   # pred accum

                # each 512-wide PSUM bank region is its own accumulation
                # group: first writer sets start=True, last sets stop=True
                for s in range(TILE_N // 512):
                    hs = slice(s * 512, (s + 1) * 512)
                    nc.tensor.matmul(p1[:, hs], cw["wa"][:], cur_t[:, hs],
                                     start=True, stop=False)
                    nc.tensor.matmul(p2[:, hs], cw["wb"][:], cur_t[:, hs],
                                     start=True, stop=False)
                    nc.tensor.matmul(p3[:, hs], cw["wi"][:], cur_t[:, hs],
                                     start=True, stop=False)
                    nc.tensor.matmul(p1[:, hs], cw["wc"][:], oh[0:16, hs],
                                     start=False, stop=True)
                    nc.tensor.matmul(p2[:, hs], cw["wd"][32:48, :], oh[32:48, hs],
                                     start=False, stop=False)
                    nc.tensor.matmul(p3[:, hs], cw["wj"][64:80, :], oh[64:80, hs],
                                     start=False, stop=False)
                    nc.tensor.matmul(p2[:, hs], cw["we"][:], nxt_t[:, hs],
                                     start=False, stop=True)

                h1 = mp.tile([FEAT, TILE_N], dt_dat, tag="h1")
                nc.scalar.activation(h1[:], p1[:], AF.Relu)
                hl2 = mp.tile([NL, TILE_N], dt_dat, tag="hl2")
                nc.scalar.activation(hl2[H2_BASE:NL, :], p2[H2_BASE:NL, :],
                                     AF.Relu)
                lg = mp.tile([ACT_TOTAL, TILE_N], f32, tag="lg")
                nc.vector.tensor_copy(lg[:], p2[0:ACT_TOTAL, :])

                for s in range(TILE_N // 512):
                    hs = slice(s * 512, (s + 1) * 512)
                    nc.tensor.matmul(p3[:, hs], cw["wf2"][:], h1[:, hs],
                                     start=False, stop=False)
                    nc.tensor.matmul(p3[:, hs], cw["wg2"][H2_BASE:NL, :],
                                     hl2[H2_BASE:NL, hs],
                                     start=False, stop=True)

                pred = mp.tile([FEAT, TILE_N], dt_out, tag="pred")
                nc.vector.tensor_copy(pred[:], p3[:])
                nc.sync.dma_start(predT[:, sl], pred[:])
                nc.sync.dma_start(logitsT[:, sl], lg[:])

    nc.compile()
    return nc


def _get_program(variant):
    if variant not in _prog_cache:
        _prog_cache[variant] = _build_program(variant)
    return _prog_cache[variant]


def _prep_inputs(inputs, variant):
    """Host-side data staging: transpose/shard/cast. Returns per-core in_maps."""
    npdt = _np_dt(variant)
    cur = np.asarray(inputs["current_feature"], np.float32).reshape(ROWS, FEAT)
    nxt = np.asarray(inputs["next_feature"], np.float32).reshape(ROWS, FEAT)
    acts = np.asarray(inputs["actions"])                       # [F, 3, A] int32

    curT = np.ascontiguousarray(cur.T).astype(npdt)            # [128, ROWS]
    nxtT = np.ascontiguousarray(nxt.T).astype(npdt)
    a3 = np.transpose(acts, (1, 0, 2)).reshape(3, ROWS)        # row t = type t
    act16 = np.zeros((16, ROWS), ml_dtypes.bfloat16)
    act16[:15] = np.repeat(a3, 5, axis=0).astype(ml_dtypes.bfloat16)
    act80 = np.ascontiguousarray(np.tile(act16, (5, 1)))       # [80, ROWS]

    w = _prep_weights(
        inputs["fc1_w"], inputs["fc1_b"], inputs["fc2_w"], inputs["fc2_b"],
        inputs["head_w0"], inputs["head_b0"], inputs["head_w1"],
        inputs["head_b1"], inputs["head_w2"], inputs["head_b2"])
    w = {k: np.ascontiguousarray(v).astype(npdt) for k, v in w.items()}
    iota80 = np.ascontiguousarray(
        np.tile(np.array([0, 1, 2, 3, 4] * 3 + [0], np.float32), 5)
    ).reshape(80, 1)

    in_maps = []
    for c in range(NCORES):
        s = slice(c * RPC, (c + 1) * RPC)
        m = {"curT": np.ascontiguousarray(curT[:, s]),
             "nxtT": np.ascontiguousarray(nxtT[:, s]),
             "act80": np.ascontiguousarray(act80[:, s]),
             "iota80": iota80}
        m.update(w)
        in_maps.append(m)
    return in_maps


def _assemble_outputs(results):
    predT = np.concatenate([np.asarray(r["predT"], np.float32)
                            for r in results], axis=1)          # [128, ROWS]
    logitsT = np.concatenate([np.asarray(r["logitsT"], np.float32)
                              for r in results], axis=1)        # [21, ROWS]
    pred = predT.T.reshape(F_FRAMES, A_AGENTS, FEAT).astype(np.float32)
    lg = logitsT.T                                              # [ROWS, 21]
    logits0 = lg[:, 0:5].reshape(F_FRAMES, A_AGENTS, 5).astype(np.float32)
    logits1 = lg[:, 5:12].reshape(F_FRAMES, A_AGENTS, 7).astype(np.float32)
    logits2 = lg[:, 12:21].reshape(F_FRAMES, A_AGENTS, 9).astype(np.float32)
    return pred, logits0, logits1, logits2


def kernel(**inputs):
    variant = VARIANT
    nc = _get_program(variant)
    in_maps = _prep_inputs(inputs, variant)
    res = run_bass_kernel_spmd(nc, in_maps, list(range(NCORES)))
    return _assemble_outputs(res.results)


# revision 5
# speedup vs baseline: 1.6617x; 1.6617x over previous
"""Trainium2 Bass kernel for nn_AdversarialHead (scatter_memory).

Computes, for F*A = 131072 rows:
  one_hot  = multi-hot(actions + starts)                      [rows, 21]
  h        = leaky_relu(fc1_w @ [cur; one_hot] + fc1_b, 0.1)  [rows, 140]
  pred     = fc2_w @ h + fc2_b                                [rows, 128]
  logits_i = head_wi @ [cur; nxt] + head_bi                   [rows, 5/7/9]

Strategy (pure data parallel over 8 cores, 16384 rows each):
- Host pre-transposes cur/nxt to [feat, rows] so every device matmul is a
  plain orientation-B matmul (features on partitions, rows on the free dim).
- The one-hot scatter becomes: per-partition is_equal against an iota
  constant (building one-hotT tiles on chip), then a small-K matmul against
  host-prebuilt embedding tables (fc1_w columns re-indexed by action value).
  All biases are folded into the row-15 "ones" row of those tables.
- leaky_relu is decomposed as 0.1*x + 0.9*relu(x); the linear 0.1*x path is
  folded into the weights (Wfuse = 0.1*fc2_w@fc1_w) so the device only needs
  a single Relu per PSUM tile; the relu path uses 0.9-scaled fc2 weights.
- f32 mode uses float32r matmuls (full-rate PE); bf16 mode halves HBM bytes.

Outputs are produced transposed ([feat, rows]) and un-transposed on host.
"""
import os

import numpy as np
import ml_dtypes

import concourse.bass as bass
import concourse.tile as tile
from concourse import bacc, mybir
from concourse.bass_utils import run_bass_kernel_spmd

# ---------------------------------------------------------------- constants
F_FRAMES, A_AGENTS, FEAT = 4096, 32, 128
NVEC = [5, 7, 9]
STARTS = [0, 5, 12]
ACT_TOTAL = 21            # sum(NVEC)
HID = 140
NCORES = 8
ROWS = F_FRAMES * A_AGENTS          # 131072
RPC = ROWS // NCORES                # 16384 rows per core
TILE_N = 1024                       # rows per megatile
NT = RPC // TILE_N                  # 16 megatiles per core
HID2 = HID - FEAT                   # 12 hidden dims beyond partition 128

# psum2 layout: rows 0:21 = logits, rows 21:32 = zero pad, rows 32:44 = h2
# (compute-engine PSUM reads must start at a 32-aligned partition)
NL = 44
H2_BASE = 32

# Variant: "f32" (float32 I/O, float32r matmuls) or "bf16"
VARIANT = os.environ.get("KERNEL_DTYPE", "f32")

_prog_cache = {}


def _np_dt(variant):
    return np.float32 if variant == "f32" else ml_dtypes.bfloat16


def _prep_weights(fc1_w, fc1_b, fc2_w, fc2_b,
                  head_w0, head_b0, head_w1, head_b1, head_w2, head_b2):
    """Build all stationary (lhsT) operands in float32. Shapes noted as [K, M]."""
    f32 = np.float32
    fc1_w = np.asarray(fc1_w, f32); fc1_b = np.asarray(fc1_b, f32)
    fc2_w = np.asarray(fc2_w, f32); fc2_b = np.asarray(fc2_b, f32)
    wcat = np.concatenate([np.asarray(head_w0, f32), np.asarray(head_w1, f32),
                           np.asarray(head_w2, f32)], axis=0)          # [21, 256]
    bcat = np.concatenate([np.asarray(head_b0, f32), np.asarray(head_b1, f32),
                           np.asarray(head_b2, f32)], axis=0)          # [21]

    wfuse = 0.1 * (fc2_w @ fc1_w)                                      # [128, 149]

    # embedding row index map: p = t*5 + v  ->  fc1 input column 128+STARTS[t]+v
    emb_cols = [FEAT + STARTS[t] + v for t in range(3) for v in range(5)]

    w = {}
    w["wa"] = fc1_w[:FEAT, :FEAT].T.copy()              # [128, 128] h1 from cur
    wb = np.zeros((FEAT, NL), f32)                      # [128, 44] logits+h2 from cur
    wb[:, :ACT_TOTAL] = wcat[:, :FEAT].T
    wb[:, H2_BASE:] = fc1_w[FEAT:, :FEAT].T
    w["wb"] = wb
    wc = np.zeros((16, FEAT), f32)                      # [16, 128] h1 from one-hot
    wc[:15, :] = fc1_w[:FEAT, emb_cols].T
    wc[15, :] = fc1_b[:FEAT]
    w["wc"] = wc
    wd = np.zeros((16, NL), f32)                        # [16, 44] h2+head biases
    wd[:15, H2_BASE:] = fc1_w[FEAT:, emb_cols].T
    wd[15, H2_BASE:] = fc1_b[FEAT:]
    wd[15, :ACT_TOTAL] = bcat
    w["wd"] = wd
    we = np.zeros((FEAT, NL), f32)                      # [128, 44] logits from nxt
    we[:, :ACT_TOTAL] = wcat[:, FEAT:].T
    w["we"] = we
    w["wi"] = wfuse[:, :FEAT].T.copy()                  # [128, 128] 0.1-path from cur
    wj = np.zeros((16, FEAT), f32)                      # [16, 128] 0.1-path from one-hot
    wj[:15, :] = wfuse[:, emb_cols].T
    wj[15, :] = 0.1 * (fc2_w @ fc1_b) + fc2_b
    w["wj"] = wj
    w["wf2"] = (0.9 * fc2_w[:, :FEAT]).T.copy()         # [128, 128] relu path h1
    w["wg2"] = (0.9 * fc2_w[:, FEAT:]).T.copy()         # [12, 128] relu path h2
    return w


def _build_program(variant):
    """Build the SPMD Bass program (identical across cores)."""
    if variant == "f32":
        dt_dat = mybir.dt.float32r     # matmul operands, stored as f32
        dt_out = mybir.dt.float32
    else:
        dt_dat = mybir.dt.bfloat16
        dt_out = mybir.dt.bfloat16
    dt_act = mybir.dt.bfloat16         # action values (0..4, exact in bf16)
    f32 = mybir.dt.float32

    AF = mybir.ActivationFunctionType
    OP = mybir.AluOpType

    nc = bacc.Bacc(None, target_bir_lowering=False, debug=False)

    def din(name, shape, dt):
        return nc.dram_tensor(name, list(shape), dt, kind="ExternalInput").ap()

    def dout(name, shape, dt):
        return nc.dram_tensor(name, list(shape), dt, kind="ExternalOutput").ap()

    curT = din("curT", (FEAT, RPC), dt_dat)
    nxtT = din("nxtT", (FEAT, RPC), dt_dat)
    act80 = din("act80", (80, RPC), dt_act)
    w_shapes = {"wa": (FEAT, FEAT), "wb": (FEAT, NL), "wc": (16, FEAT),
                "wd": (16, NL), "we": (FEAT, NL), "wi": (FEAT, FEAT),
                "wj": (16, FEAT), "wf2": (FEAT, FEAT), "wg2": (HID2, FEAT)}
    # wd / wj / wg2 are loaded at 32-aligned partition offsets so their
    # small-K matmuls occupy distinct PE row groups
    w_dram = {k: din(k, s, dt_dat) for k, s in w_shapes.items()}
    iota80 = din("iota80", (80, 1), f32)
    predT = dout("predT", (FEAT, RPC), dt_out)
    logitsT = dout("logitsT", (ACT_TOTAL, RPC), f32)

    with tile.TileContext(nc) as tc:
        with (
            tc.tile_pool(name="consts", bufs=1) as cp,
            tc.tile_pool(name="io", bufs=3) as iop,
            tc.tile_pool(name="mid", bufs=3) as mp,
            tc.tile_pool(name="pp1", bufs=1, space="PSUM") as pp1,
            tc.tile_pool(name="pp2", bufs=2, space="PSUM") as pp2,
            tc.tile_pool(name="pp3", bufs=1, space="PSUM") as pp3,
        ):
            # ---- load constants once ----
            cw = {}
            for k in ("wa", "wb", "wc", "we", "wi", "wf2"):
                t = cp.tile(w_shapes[k], dt_dat, name=f"c_{k}")
                nc.sync.dma_start(t[:], w_dram[k])
                cw[k] = t
            t = cp.tile([NL, FEAT], dt_dat, name="c_wg2")
            nc.sync.dma_start(t[H2_BASE:NL, :], w_dram["wg2"])
            cw["wg2"] = t
            # wd lives at partitions 32..47, wj at 64..79 so their small-K
            # matmuls land in distinct PE row-groups and run concurrently.
            t = cp.tile([48, NL], dt_dat, name="c_wd")
            nc.sync.dma_start(t[32:48, :], w_dram["wd"])
            cw["wd"] = t
            t = cp.tile([80, FEAT], dt_dat, name="c_wj")
            nc.sync.dma_start(t[64:80, :], w_dram["wj"])
            cw["wj"] = t
            iota_t = cp.tile([80, 1], f32, name="c_iota")
            nc.sync.dma_start(iota_t[:], iota80)

            for it in range(NT):
                sl = slice(it * TILE_N, (it + 1) * TILE_N)
                cur_t = iop.tile([FEAT, TILE_N], dt_dat, tag="cur")
                nc.sync.dma_start(cur_t[:], curT[:, sl])
                nxt_t = iop.tile([FEAT, TILE_N], dt_dat, tag="nxt")
                nc.sync.dma_start(nxt_t[:], nxtT[:, sl])
                act_t = iop.tile([80, TILE_N], dt_act, tag="act")
                nc.sync.dma_start(act_t[:], act80[:, sl])

                # one-hotT tiles (replicated in three row-groups; row15 of each
                # group is all-ones because the action row is 0 and iota is 0)
                oh = mp.tile([80, TILE_N], dt_dat, tag="oh")
                nc.vector.tensor_scalar(oh[:], act_t[:], iota_t[:], None,
                                        OP.is_equal)

                p1 = pp1.tile([FEAT, TILE_N], f32, tag="p1")   # h1 pre-act
                p2 = pp2.tile([NL, TILE_N], f32, tag="p2")     # h2 + logits
                p3 = pp3.tile([FEAT, TILE_N], f32, tag="p3")   # pred accum

                # each 512-wide PSUM bank region is its own accumulation
                # group: first writer sets start=True, last sets stop=True
                for s in range(TILE_N // 512):
                    hs = slice(s * 512, (s + 1) * 512)
                    nc.tensor.matmul(p1[:, hs], cw["wa"][:], cur_t[:, hs],
                                     start=True, stop=False)
                    nc.tensor.matmul(p2[:, hs], cw["wb"][:], cur_t[:, hs],
                                     start=True, stop=False)
                    nc.tensor.matmul(p3[:, hs], cw["wi"][:], cur_t[:, hs],
                                     start=True, stop=False)
                    nc.tensor.matmul(p1[:, hs], cw["wc"][:], oh[0:16, hs],
                                     start=False, stop=True)
                    nc.tensor.matmul(p2[:, hs], cw["wd"][32:48, :], oh[32:48, hs],
                                     start=False, stop=False)
                    nc.tensor.matmul(p3[:, hs], cw["wj"][64:80, :], oh[64:80, hs],
                                     start=False, stop=False)
                    nc.tensor.matmul(p2[:, hs], cw["we"][:], nxt_t[:, hs],
                                     start=False, stop=True)

                h1 = mp.tile([FEAT, TILE_N], dt_dat, tag="h1")
                nc.scalar.activation(h1[:], p1[:], AF.Relu)
                hl2 = mp.tile([NL, TILE_N], dt_dat, tag="hl2")
                nc.scalar.activation(hl2[H2_BASE:NL, :], p2[H2_BASE:NL, :],
                                     AF.Relu)
                lg = mp.tile([ACT_TOTAL, TILE_N], f32, tag="lg")
                nc.vector.tensor_copy(lg[:], p2[0:ACT_TOTAL, :])

                for s in range(TILE_N // 512):
                    hs = slice(s * 512, (s + 1) * 512)
                    nc.tensor.matmul(p3[:, hs], cw["wf2"][:], h1[:, hs],
                                     start=False, stop=False)
                    nc.tensor.matmul(p3[:, hs], cw["wg2"][H2_BASE:NL, :],
                                     hl2[H2_BASE:NL, hs],
                                     start=False, stop=True)

                pred = mp.tile([FEAT, TILE_N], dt_out, tag="pred")
                nc.vector.tensor_copy(pred[:], p3[:])
                nc.sync.dma_start(predT[:, sl], pred[:])
                nc.sync.dma_start(logitsT[:, sl], lg[:])

    nc.compile()
    return nc


def _get_program(variant):
    if variant not in _prog_cache:
        _prog_cache[variant] = _build_program(variant)
    return _prog_cache[variant]


def _prep_inputs(inputs, variant):
    """Host-side data staging: transpose/shard/cast. Returns per-core in_maps."""
    npdt = _np_dt(variant)
    cur = np.asarray(inputs["current_feature"], np.float32).reshape(ROWS, FEAT)
    nxt = np.asarray(inputs["next_feature"], np.float32).reshape(ROWS, FEAT)
    acts = np.asarray(inputs["actions"])                       # [F, 3, A] int32

    curT = np.ascontiguousarray(cur.T).astype(npdt)            # [128, ROWS]
    nxtT = np.ascontiguousarray(nxt.T).astype(npdt)
    a3 = np.transpose(acts, (1, 0, 2)).reshape(3, ROWS)        # row t = type t
    act16 = np.zeros((16, ROWS), ml_dtypes.bfloat16)
    act16[:15] = np.repeat(a3, 5, axis=0).astype(ml_dtypes.bfloat16)
    act80 = np.ascontiguousarray(np.tile(act16, (5, 1)))       # [80, ROWS]

    w = _prep_weights(
        inputs["fc1_w"], inputs["fc1_b"], inputs["fc2_w"], inputs["fc2_b"],
        inputs["head_w0"], inputs["head_b0"], inputs["head_w1"],
        inputs["head_b1"], inputs["head_w2"], inputs["head_b2"])
    w = {k: np.ascontiguousarray(v).astype(npdt) for k, v in w.items()}
    iota80 = np.ascontiguousarray(
        np.tile(np.array([0, 1, 2, 3, 4] * 3 + [0], np.float32), 5)
    ).reshape(80, 1)

    in_maps = []
    for c in range(NCORES):
        s = slice(c * RPC, (c + 1) * RPC)
        m = {"curT": np.ascontiguousarray(curT[:, s]),
             "nxtT": np.ascontiguousarray(nxtT[:, s]),
             "act80": np.ascontiguousarray(act80[:, s]),
             "iota80": iota80}
        m.update(w)
        in_maps.append(m)
    return in_maps


def _assemble_outputs(results):
    predT = np.concatenate([np.asarray(r["predT"], np.float32)
                            for r in results], axis=1)          # [128, ROWS]
    logitsT = np.concatenate([np.asarray(r["logitsT"], np.float32)
                              for r in results], axis=1)        # [21, ROWS]
    pred = predT.T.reshape(F_FRAMES, A_AGENTS, FEAT).astype(np.float32)
    lg = logitsT.T                                              # [ROWS, 21]
    logits0 = lg[:, 0:5].reshape(F_FRAMES, A_AGENTS, 5).astype(np.float32)
    logits1 = lg[:, 5:12].reshape(F_FRAMES, A_AGENTS, 7).astype(np.float32)
    logits2 = lg[:, 12:21].reshape(F_FRAMES, A_AGENTS, 9).astype(np.float32)
    return pred, logits0, logits1, logits2


def kernel(**inputs):
    variant = VARIANT
    nc = _get_program(variant)
    in_maps = _prep_inputs(inputs, variant)
    res = run_bass_kernel_spmd(nc, in_maps, list(range(NCORES)))
    return _assemble_outputs(res.results)


# revision 7
# speedup vs baseline: 1.6635x; 1.0011x over previous
"""Trainium2 Bass kernel for nn_AdversarialHead (scatter_memory).

Computes, for F*A = 131072 rows:
  one_hot  = multi-hot(actions + starts)                      [rows, 21]
  h        = leaky_relu(fc1_w @ [cur; one_hot] + fc1_b, 0.1)  [rows, 140]
  pred     = fc2_w @ h + fc2_b                                [rows, 128]
  logits_i = head_wi @ [cur; nxt] + head_bi                   [rows, 5/7/9]

Strategy (pure data parallel over 8 cores, 16384 rows each):
- Host pre-transposes cur/nxt to [feat, rows] so every device matmul is a
  plain orientation-B matmul (features on partitions, rows on the free dim).
- The one-hot scatter becomes: per-partition is_equal against an iota
  constant (building one-hotT tiles on chip), then a small-K matmul against
  host-prebuilt embedding tables (fc1_w columns re-indexed by action value).
  All biases are folded into the row-15 "ones" row of those tables.
- leaky_relu is decomposed as 0.1*x + 0.9*relu(x); the linear 0.1*x path is
  folded into the weights (Wfuse = 0.1*fc2_w@fc1_w) so the device only needs
  a single Relu per PSUM tile; the relu path uses 0.9-scaled fc2 weights.
- f32 mode uses float32r matmuls (full-rate PE); bf16 mode halves HBM bytes.

Outputs are produced transposed ([feat, rows]) and un-transposed on host.
"""
import os

import numpy as np
import ml_dtypes

import concourse.bass as bass
import concourse.tile as tile
from concourse import bacc, mybir
from concourse.bass_utils import run_bass_kernel_spmd

# ---------------------------------------------------------------- constants
F_FRAMES, A_AGENTS, FEAT = 4096, 32, 128
NVEC = [5, 7, 9]
STARTS = [0, 5, 12]
ACT_TOTAL = 21            # sum(NVEC)
HID = 140
NCORES = 8
ROWS = F_FRAMES * A_AGENTS          # 131072
RPC = ROWS // NCORES                # 16384 rows per core
TILE_N = 1024                       # rows per megatile
NT = RPC // TILE_N                  # 16 megatiles per core
HID2 = HID - FEAT                   # 12 hidden dims beyond partition 128

# psum2 layout: rows 0:21 = logits, rows 21:32 = zero pad, rows 32:44 = h2
# (compute-engine PSUM reads must start at a 32-aligned partition)
NL = 44
H2_BASE = 32

# Variant: "f32" (float32 I/O, float32r matmuls) or "bf16"
VARIANT = os.environ.get("KERNEL_DTYPE", "f32")

_prog_cache = {}


def _np_dt(variant):
    return np.float32 if variant == "f32" else ml_dtypes.bfloat16


def _prep_weights(fc1_w, fc1_b, fc2_w, fc2_b,
                  head_w0, head_b0, head_w1, head_b1, head_w2, head_b2):
    """Build all stationary (lhsT) operands in float32. Shapes noted as [K, M]."""
    f32 = np.float32
    fc1_w = np.asarray(fc1_w, f32); fc1_b = np.asarray(fc1_b, f32)
    fc2_w = np.asarray(fc2_w, f32); fc2_b = np.asarray(fc2_b, f32)
    wcat = np.concatenate([np.asarray(head_w0, f32), np.asarray(head_w1, f32),
                           np.asarray(head_w2, f32)], axis=0)          # [21, 256]
    bcat = np.concatenate([np.asarray(head_b0, f32), np.asarray(head_b1, f32),
                           np.asarray(head_b2, f32)], axis=0)          # [21]

    wfuse = 0.1 * (fc2_w @ fc1_w)                                      # [128, 149]

    # embedding row index map: p = t*5 + v  ->  fc1 input column 128+STARTS[t]+v
    emb_cols = [FEAT + STARTS[t] + v for t in range(3) for v in range(5)]

    w = {}
    w["wa"] = fc1_w[:FEAT, :FEAT].T.copy()              # [128, 128] h1 from cur
    wb = np.zeros((FEAT, NL), f32)                      # [128, 44] logits+h2 from cur
    wb[:, :ACT_TOTAL] = wcat[:, :FEAT].T
    wb[:, H2_BASE:] = fc1_w[FEAT:, :FEAT].T
    w["wb"] = wb
    wc = np.zeros((16, FEAT), f32)                      # [16, 128] h1 from one-hot
    wc[:15, :] = fc1_w[:FEAT, emb_cols].T
    wc[15, :] = fc1_b[:FEAT]
    w["wc"] = wc
    wd = np.zeros((16, NL), f32)                        # [16, 44] h2+head biases
    wd[:15, H2_BASE:] = fc1_w[FEAT:, emb_cols].T
    wd[15, H2_BASE:] = fc1_b[FEAT:]
    wd[15, :ACT_TOTAL] = bcat
    w["wd"] = wd
    we = np.zeros((FEAT, NL), f32)                      # [128, 44] logits from nxt
    we[:, :ACT_TOTAL] = wcat[:, FEAT:].T
    w["we"] = we
    w["wi"] = wfuse[:, :FEAT].T.copy()                  # [128, 128] 0.1-path from cur
    wj = np.zeros((16, FEAT), f32)                      # [16, 128] 0.1-path from one-hot
    wj[:15, :] = wfuse[:, emb_cols].T
    wj[15, :] = 0.1 * (fc2_w @ fc1_b) + fc2_b
    w["wj"] = wj
    w["wf2"] = (0.9 * fc2_w[:, :FEAT]).T.copy()         # [128, 128] relu path h1
    w["wg2"] = (0.9 * fc2_w[:, FEAT:]).T.copy()         # [12, 128] relu path h2
    return w


def _build_program(variant):
    """Build the SPMD Bass program (identical across cores)."""
    if variant == "f32":
        dt_dat = mybir.dt.float32r     # matmul operands, stored as f32
        dt_out = mybir.dt.float32
    else:
        dt_dat = mybir.dt.bfloat16
        dt_out = mybir.dt.bfloat16
    dt_act = mybir.dt.bfloat16         # action values (0..4, exact in bf16)
    f32 = mybir.dt.float32

    AF = mybir.ActivationFunctionType
    OP = mybir.AluOpType

    nc = bacc.Bacc(None, target_bir_lowering=False, debug=False)

    def din(name, shape, dt):
        return nc.dram_tensor(name, list(shape), dt, kind="ExternalInput").ap()

    def dout(name, shape, dt):
        return nc.dram_tensor(name, list(shape), dt, kind="ExternalOutput").ap()

    curT = din("curT", (FEAT, RPC), dt_dat)
    nxtT = din("nxtT", (FEAT, RPC), dt_dat)
    act80 = din("act80", (80, RPC), dt_act)
    w_shapes = {"wa": (FEAT, FEAT), "wb": (FEAT, NL), "wc": (16, FEAT),
                "wd": (16, NL), "we": (FEAT, NL), "wi": (FEAT, FEAT),
                "wj": (16, FEAT), "wf2": (FEAT, FEAT), "wg2": (HID2, FEAT)}
    # wd / wj / wg2 are loaded at 32-aligned partition offsets so their
    # small-K matmuls occupy distinct PE row groups
    w_dram = {k: din(k, s, dt_dat) for k, s in w_shapes.items()}
    iota80 = din("iota80", (80, 1), f32)
    predT = dout("predT", (FEAT, RPC), dt_out)
    logitsT = dout("logitsT", (ACT_TOTAL, RPC), f32)

    with tile.TileContext(nc) as tc:
        with (
            tc.tile_pool(name="consts", bufs=1) as cp,
            tc.tile_pool(name="io", bufs=3) as iop,
            tc.tile_pool(name="mid", bufs=3) as mp,
            tc.tile_pool(name="pp1", bufs=2, space="PSUM") as pp1,
            tc.tile_pool(name="pp2", bufs=1, space="PSUM") as pp2,
            tc.tile_pool(name="pp3", bufs=1, space="PSUM") as pp3,
        ):
            # ---- load constants once ----
            cw = {}
            for k in ("wa", "wb", "wc", "we", "wi", "wf2"):
                t = cp.tile(w_shapes[k], dt_dat, name=f"c_{k}")
                nc.sync.dma_start(t[:], w_dram[k])
                cw[k] = t
            t = cp.tile([NL, FEAT], dt_dat, name="c_wg2")
            nc.sync.dma_start(t[H2_BASE:NL, :], w_dram["wg2"])
            cw["wg2"] = t
            # wd lives at partitions 32..47, wj at 64..79 so their small-K
            # matmuls land in distinct PE row-groups and run concurrently.
            t = cp.tile([48, NL], dt_dat, name="c_wd")
            nc.sync.dma_start(t[32:48, :], w_dram["wd"])
            cw["wd"] = t
            t = cp.tile([80, FEAT], dt_dat, name="c_wj")
            nc.sync.dma_start(t[64:80, :], w_dram["wj"])
            cw["wj"] = t
            iota_t = cp.tile([80, 1], f32, name="c_iota")
            nc.sync.dma_start(iota_t[:], iota80)

            for it in range(NT):
                sl = slice(it * TILE_N, (it + 1) * TILE_N)
                cur_t = iop.tile([FEAT, TILE_N], dt_dat, tag="cur")
                nc.sync.dma_start(cur_t[:], curT[:, sl])
                nxt_t = iop.tile([FEAT, TILE_N], dt_dat, tag="nxt")
                nc.sync.dma_start(nxt_t[:], nxtT[:, sl])
                act_t = iop.tile([80, TILE_N], dt_act, tag="act")
                nc.sync.dma_start(act_t[:], act80[:, sl])

                # one-hotT tiles (replicated in three row-groups; row15 of each
                # group is all-ones because the action row is 0 and iota is 0)
                oh = mp.tile([80, TILE_N], dt_dat, tag="oh")
                nc.vector.tensor_scalar(oh[:], act_t[:], iota_t[:], None,
                                        OP.is_equal)

                p1 = pp1.tile([FEAT, TILE_N], f32, tag="p1")   # h1 pre-act
                p2 = pp2.tile([NL, TILE_N], f32, tag="p2")     # h2 + logits
                p3 = pp3.tile([FEAT, TILE_N], f32, tag="p3")   # pred accum

                # each 512-wide PSUM bank region is its own accumulation
                # group: first writer sets start=True, last sets stop=True.
                # MMs are grouped by PSUM bank — consecutive bank switches
                # micro-idle the PE and keep the HAM clock gate cold.
                for s in range(TILE_N // 512):
                    hs = slice(s * 512, (s + 1) * 512)
                    nc.tensor.matmul(p1[:, hs], cw["wa"][:], cur_t[:, hs],
                                     start=True, stop=False)
                    nc.tensor.matmul(p1[:, hs], cw["wc"][:], oh[0:16, hs],
                                     start=False, stop=True)
                for s in range(TILE_N // 512):
                    hs = slice(s * 512, (s + 1) * 512)
                    nc.tensor.matmul(p2[:, hs], cw["wb"][:], cur_t[:, hs],
                                     start=True, stop=False)
                    nc.tensor.matmul(p2[:, hs], cw["wd"][32:48, :], oh[32:48, hs],
                                     start=False, stop=False)
                    nc.tensor.matmul(p2[:, hs], cw["we"][:], nxt_t[:, hs],
                                     start=False, stop=True)
                for s in range(TILE_N // 512):
                    hs = slice(s * 512, (s + 1) * 512)
                    nc.tensor.matmul(p3[:, hs], cw["wi"][:], cur_t[:, hs],
                                     start=True, stop=False)
                    nc.tensor.matmul(p3[:, hs], cw["wj"][64:80, :], oh[64:80, hs],
                                     start=False, stop=False)

                h1 = mp.tile([FEAT, TILE_N], dt_dat, tag="h1")
                nc.scalar.activation(h1[:], p1[:], AF.Relu)
                hl2 = mp.tile([NL, TILE_N], dt_dat, tag="hl2")
                nc.scalar.activation(hl2[H2_BASE:NL, :], p2[H2_BASE:NL, :],
                                     AF.Relu)
                lg = mp.tile([ACT_TOTAL, TILE_N], f32, tag="lg")
                nc.vector.tensor_copy(lg[:], p2[0:ACT_TOTAL, :])

                for s in range(TILE_N // 512):
                    hs = slice(s * 512, (s + 1) * 512)
                    nc.tensor.matmul(p3[:, hs], cw["wf2"][:], h1[:, hs],
                                     start=False, stop=False)
                    nc.tensor.matmul(p3[:, hs], cw["wg2"][H2_BASE:NL, :],
                                     hl2[H2_BASE:NL, hs],
                                     start=False, stop=True)

                pred = mp.tile([FEAT, TILE_N], dt_out, tag="pred")
                nc.vector.tensor_copy(pred[:], p3[:])
                nc.sync.dma_start(predT[:, sl], pred[:])
                nc.sync.dma_start(logitsT[:, sl], lg[:])

    nc.compile()
    return nc


def _get_program(variant):
    if variant not in _prog_cache:
        _prog_cache[variant] = _build_program(variant)
    return _prog_cache[variant]


def _prep_inputs(inputs, variant):
    """Host-side data staging: transpose/shard/cast. Returns per-core in_maps."""
    npdt = _np_dt(variant)
    cur = np.asarray(inputs["current_feature"], np.float32).reshape(ROWS, FEAT)
    nxt = np.asarray(inputs["next_feature"], np.float32).reshape(ROWS, FEAT)
    acts = np.asarray(inputs["actions"])                       # [F, 3, A] int32

    curT = np.ascontiguousarray(cur.T).astype(npdt)            # [128, ROWS]
    nxtT = np.ascontiguousarray(nxt.T).astype(npdt)
    a3 = np.transpose(acts, (1, 0, 2)).reshape(3, ROWS)        # row t = type t
    act16 = np.zeros((16, ROWS), ml_dtypes.bfloat16)
    act16[:15] = np.repeat(a3, 5, axis=0).astype(ml_dtypes.bfloat16)
    act80 = np.ascontiguousarray(np.tile(act16, (5, 1)))       # [80, ROWS]

    w = _prep_weights(
        inputs["fc1_w"], inputs["fc1_b"], inputs["fc2_w"], inputs["fc2_b"],
        inputs["head_w0"], inputs["head_b0"], inputs["head_w1"],
        inputs["head_b1"], inputs["head_w2"], inputs["head_b2"])
    w = {k: np.ascontiguousarray(v).astype(npdt) for k, v in w.items()}
    iota80 = np.ascontiguousarray(
        np.tile(np.array([0, 1, 2, 3, 4] * 3 + [0], np.float32), 5)
    ).reshape(80, 1)

    in_maps = []
    for c in range(NCORES):
        s = slice(c * RPC, (c + 1) * RPC)
        m = {"curT": np.ascontiguousarray(curT[:, s]),
             "nxtT": np.ascontiguousarray(nxtT[:, s]),
             "act80": np.ascontiguousarray(act80[:, s]),
             "iota80": iota80}
        m.update(w)
        in_maps.append(m)
    return in_maps


def _assemble_outputs(results):
    predT = np.concatenate([np.asarray(r["predT"], np.float32)
                            for r in results], axis=1)          # [128, ROWS]
    logitsT = np.concatenate([np.asarray(r["logitsT"], np.float32)
                              for r in results], axis=1)        # [21, ROWS]
    pred = predT.T.reshape(F_FRAMES, A_AGENTS, FEAT).astype(np.float32)
    lg = logitsT.T                                              # [ROWS, 21]
    logits0 = lg[:, 0:5].reshape(F_FRAMES, A_AGENTS, 5).astype(np.float32)
    logits1 = lg[:, 5:12].reshape(F_FRAMES, A_AGENTS, 7).astype(np.float32)
    logits2 = lg[:, 12:21].reshape(F_FRAMES, A_AGENTS, 9).astype(np.float32)
    return pred, logits0, logits1, logits2


def kernel(**inputs):
    variant = VARIANT
    nc = _get_program(variant)
    in_maps = _prep_inputs(inputs, variant)
    res = run_bass_kernel_spmd(nc, in_maps, list(range(NCORES)))
    return _assemble_outputs(res.results)


# revision 8
# speedup vs baseline: 2.6473x; 1.5914x over previous
"""Trainium2 Bass kernel for nn_AdversarialHead (scatter_memory).

Computes, for F*A = 131072 rows:
  one_hot  = multi-hot(actions + starts)                      [rows, 21]
  h        = leaky_relu(fc1_w @ [cur; one_hot] + fc1_b, 0.1)  [rows, 140]
  pred     = fc2_w @ h + fc2_b                                [rows, 128]
  logits_i = head_wi @ [cur; nxt] + head_bi                   [rows, 5/7/9]

Strategy (pure data parallel over 8 cores, 16384 rows each):
- Host pre-transposes cur/nxt to [feat, rows] so every device matmul is a
  plain orientation-B matmul (features on partitions, rows on the free dim).
- The one-hot scatter becomes: per-partition is_equal against an iota
  constant (building one-hotT tiles on chip), then a small-K matmul against
  host-prebuilt embedding tables (fc1_w columns re-indexed by action value).
  All biases are folded into the row-15 "ones" row of those tables.
- leaky_relu is decomposed as 0.1*x + 0.9*relu(x); the linear 0.1*x path is
  folded into the weights (Wfuse = 0.1*fc2_w@fc1_w) so the device only needs
  a single Relu per PSUM tile; the relu path uses 0.9-scaled fc2 weights.
- f32 mode uses float32r matmuls (full-rate PE); bf16 mode halves HBM bytes.

Outputs are produced transposed ([feat, rows]) and un-transposed on host.
"""
import os

import numpy as np
import ml_dtypes

import concourse.bass as bass
import concourse.tile as tile
from concourse import bacc, mybir
from concourse.bass_utils import run_bass_kernel_spmd

# ---------------------------------------------------------------- constants
F_FRAMES, A_AGENTS, FEAT = 4096, 32, 128
NVEC = [5, 7, 9]
STARTS = [0, 5, 12]
ACT_TOTAL = 21            # sum(NVEC)
HID = 140
NCORES = 8
ROWS = F_FRAMES * A_AGENTS          # 131072
RPC = ROWS // NCORES                # 16384 rows per core
TILE_N = 1024                       # rows per megatile
NT = RPC // TILE_N                  # 16 megatiles per core
HID2 = HID - FEAT                   # 12 hidden dims beyond partition 128

# psum2 layout: rows 0:21 = logits, rows 21:32 = zero pad, rows 32:44 = h2
# (compute-engine PSUM reads must start at a 32-aligned partition)
NL = 44
H2_BASE = 32

# Variant: "f32" (float32 I/O, float32r matmuls) or "bf16"
VARIANT = os.environ.get("KERNEL_DTYPE", "f32")

_prog_cache = {}


def _np_dt(variant):
    return np.float32 if variant == "f32" else ml_dtypes.bfloat16


def _prep_weights(fc1_w, fc1_b, fc2_w, fc2_b,
                  head_w0, head_b0, head_w1, head_b1, head_w2, head_b2):
    """Build all stationary (lhsT) operands in float32. Shapes noted as [K, M]."""
    f32 = np.float32
    fc1_w = np.asarray(fc1_w, f32); fc1_b = np.asarray(fc1_b, f32)
    fc2_w = np.asarray(fc2_w, f32); fc2_b = np.asarray(fc2_b, f32)
    wcat = np.concatenate([np.asarray(head_w0, f32), np.asarray(head_w1, f32),
                           np.asarray(head_w2, f32)], axis=0)          # [21, 256]
    bcat = np.concatenate([np.asarray(head_b0, f32), np.asarray(head_b1, f32),
                           np.asarray(head_b2, f32)], axis=0)          # [21]

    wfuse = 0.1 * (fc2_w @ fc1_w)                                      # [128, 149]

    # embedding row index map: p = t*5 + v  ->  fc1 input column 128+STARTS[t]+v
    emb_cols = [FEAT + STARTS[t] + v for t in range(3) for v in range(5)]

    # Every lhsT is zero-padded to [128, 128]: uniform full-array matmuls
    # keep the PE pipeline dense (small-K / partial-M matmuls measured 2.8x
    # slower and prevent the clock gate from warming). Zero weight rows make
    # the padded contraction exact as long as the rhs rows are finite.
    w = {k: np.zeros((FEAT, FEAT), f32) for k in
         ("wa", "wb", "wc", "wd", "we", "wi", "wj", "wf2", "wg2")}
    w["wa"][:, :] = fc1_w[:FEAT, :FEAT].T               # h1 from cur
    w["wb"][:, :ACT_TOTAL] = wcat[:, :FEAT].T           # logits+h2 from cur
    w["wb"][:, H2_BASE:H2_BASE + HID2] = fc1_w[FEAT:, :FEAT].T
    w["wc"][:15, :] = fc1_w[:FEAT, emb_cols].T          # h1 from one-hot
    w["wc"][15, :] = fc1_b[:FEAT]
    w["wd"][:15, H2_BASE:H2_BASE + HID2] = fc1_w[FEAT:, emb_cols].T
    w["wd"][15, H2_BASE:H2_BASE + HID2] = fc1_b[FEAT:]
    w["wd"][15, :ACT_TOTAL] = bcat                      # h2 + all biases
    w["we"][:, :ACT_TOTAL] = wcat[:, FEAT:].T           # logits from nxt
    w["wi"][:, :] = wfuse[:, :FEAT].T                   # 0.1-path from cur
    w["wj"][:15, :] = wfuse[:, emb_cols].T              # 0.1-path from one-hot
    w["wj"][15, :] = 0.1 * (fc2_w @ fc1_b) + fc2_b
    w["wf2"][:, :] = (0.9 * fc2_w[:, :FEAT]).T          # relu path h1
    w["wg2"][H2_BASE:H2_BASE + HID2, :] = (0.9 * fc2_w[:, FEAT:]).T
    return w


def _build_program(variant):
    """Build the SPMD Bass program (identical across cores)."""
    if variant == "f32":
        dt_dat = mybir.dt.float32r     # matmul operands, stored as f32
        dt_out = mybir.dt.float32
    else:
        dt_dat = mybir.dt.bfloat16
        dt_out = mybir.dt.bfloat16
    dt_act = mybir.dt.bfloat16         # action values (0..4, exact in bf16)
    f32 = mybir.dt.float32

    AF = mybir.ActivationFunctionType
    OP = mybir.AluOpType

    nc = bacc.Bacc(None, target_bir_lowering=False, debug=False)

    def din(name, shape, dt):
        return nc.dram_tensor(name, list(shape), dt, kind="ExternalInput").ap()

    def dout(name, shape, dt):
        return nc.dram_tensor(name, list(shape), dt, kind="ExternalOutput").ap()

    curT = din("curT", (FEAT, RPC), dt_dat)
    nxtT = din("nxtT", (FEAT, RPC), dt_dat)
    act128 = din("act128", (128, RPC), dt_act)
    w_names = ("wa", "wb", "wc", "wd", "we", "wi", "wj", "wf2", "wg2")
    w_dram = {k: din(k, (FEAT, FEAT), dt_dat) for k in w_names}
    iota128 = din("iota128", (128, 1), f32)
    predT = dout("predT", (FEAT, RPC), dt_out)
    logitsT = dout("logitsT", (ACT_TOTAL, RPC), f32)

    with tile.TileContext(nc) as tc:
        with (
            tc.tile_pool(name="consts", bufs=1) as cp,
            tc.tile_pool(name="io", bufs=3) as iop,
            tc.tile_pool(name="mid", bufs=3) as mp,
            tc.tile_pool(name="pp1", bufs=2, space="PSUM") as pp1,
            tc.tile_pool(name="pp2", bufs=1, space="PSUM") as pp2,
            tc.tile_pool(name="pp3", bufs=1, space="PSUM") as pp3,
        ):
            # ---- load constants once ----
            cw = {}
            for k in w_names:
                t = cp.tile([FEAT, FEAT], dt_dat, name=f"c_{k}")
                nc.sync.dma_start(t[:], w_dram[k])
                cw[k] = t
            iota_t = cp.tile([128, 1], f32, name="c_iota")
            nc.sync.dma_start(iota_t[:], iota128)

            for it in range(NT):
                sl = slice(it * TILE_N, (it + 1) * TILE_N)
                cur_t = iop.tile([FEAT, TILE_N], dt_dat, tag="cur")
                nc.sync.dma_start(cur_t[:], curT[:, sl])
                nxt_t = iop.tile([FEAT, TILE_N], dt_dat, tag="nxt")
                nc.sync.dma_start(nxt_t[:], nxtT[:, sl])
                act_t = iop.tile([128, TILE_N], dt_act, tag="act")
                nc.sync.dma_start(act_t[:], act128[:, sl])

                # one-hotT tile (row 15 of each 16-row group is all-ones
                # because the action row is 0 and iota is 0 there)
                oh = mp.tile([128, TILE_N], dt_dat, tag="oh")
                nc.vector.tensor_scalar(oh[:], act_t[:], iota_t[:], None,
                                        OP.is_equal)

                p1 = pp1.tile([FEAT, TILE_N], f32, tag="p1")   # h1 pre-act
                p2 = pp2.tile([FEAT, TILE_N], f32, tag="p2")   # logits + h2
                p3 = pp3.tile([FEAT, TILE_N], f32, tag="p3")   # pred accum

                # each 512-wide PSUM bank region is its own accumulation
                # group: first writer sets start=True, last sets stop=True.
                # MMs are grouped by PSUM bank — consecutive bank switches
                # micro-idle the PE and keep the HAM clock gate cold.
                for s in range(TILE_N // 512):
                    hs = slice(s * 512, (s + 1) * 512)
                    nc.tensor.matmul(p1[:, hs], cw["wa"][:], cur_t[:, hs],
                                     start=True, stop=False)
                    nc.tensor.matmul(p1[:, hs], cw["wc"][:], oh[:, hs],
                                     start=False, stop=True)
                for s in range(TILE_N // 512):
                    hs = slice(s * 512, (s + 1) * 512)
                    nc.tensor.matmul(p2[:, hs], cw["wb"][:], cur_t[:, hs],
                                     start=True, stop=False)
                    nc.tensor.matmul(p2[:, hs], cw["wd"][:], oh[:, hs],
                                     start=False, stop=False)
                    nc.tensor.matmul(p2[:, hs], cw["we"][:], nxt_t[:, hs],
                                     start=False, stop=True)
                for s in range(TILE_N // 512):
                    hs = slice(s * 512, (s + 1) * 512)
                    nc.tensor.matmul(p3[:, hs], cw["wi"][:], cur_t[:, hs],
                                     start=True, stop=False)
                    nc.tensor.matmul(p3[:, hs], cw["wj"][:], oh[:, hs],
                                     start=False, stop=False)

                h1 = mp.tile([FEAT, TILE_N], dt_dat, tag="h1")
                nc.scalar.activation(h1[:], p1[:], AF.Relu)
                hl2 = mp.tile([FEAT, TILE_N], dt_dat, tag="hl2")
                nc.scalar.activation(hl2[:], p2[:], AF.Relu)
                lg = mp.tile([ACT_TOTAL, TILE_N], f32, tag="lg")
                nc.vector.tensor_copy(lg[:], p2[0:ACT_TOTAL, :])

                for s in range(TILE_N // 512):
                    hs = slice(s * 512, (s + 1) * 512)
                    nc.tensor.matmul(p3[:, hs], cw["wf2"][:], h1[:, hs],
                                     start=False, stop=False)
                    nc.tensor.matmul(p3[:, hs], cw["wg2"][:], hl2[:, hs],
                                     start=False, stop=True)

                pred = mp.tile([FEAT, TILE_N], dt_out, tag="pred")
                nc.vector.tensor_copy(pred[:], p3[:])
                nc.sync.dma_start(predT[:, sl], pred[:])
                nc.sync.dma_start(logitsT[:, sl], lg[:])

    nc.compile()
    return nc


def _get_program(variant):
    if variant not in _prog_cache:
        _prog_cache[variant] = _build_program(variant)
    return _prog_cache[variant]


def _prep_inputs(inputs, variant):
    """Host-side data staging: transpose/shard/cast. Returns per-core in_maps."""
    npdt = _np_dt(variant)
    cur = np.asarray(inputs["current_feature"], np.float32).reshape(ROWS, FEAT)
    nxt = np.asarray(inputs["next_feature"], np.float32).reshape(ROWS, FEAT)
    acts = np.asarray(inputs["actions"])                       # [F, 3, A] int32

    curT = np.ascontiguousarray(cur.T).astype(npdt)            # [128, ROWS]
    nxtT = np.ascontiguousarray(nxt.T).astype(npdt)
    a3 = np.transpose(acts, (1, 0, 2)).reshape(3, ROWS)        # row t = type t
    act16 = np.zeros((16, ROWS), ml_dtypes.bfloat16)
    act16[:15] = np.repeat(a3, 5, axis=0).astype(ml_dtypes.bfloat16)
    act128 = np.ascontiguousarray(np.tile(act16, (8, 1)))      # [128, ROWS]

    w = _prep_weights(
        inputs["fc1_w"], inputs["fc1_b"], inputs["fc2_w"], inputs["fc2_b"],
        inputs["head_w0"], inputs["head_b0"], inputs["head_w1"],
        inputs["head_b1"], inputs["head_w2"], inputs["head_b2"])
    w = {k: np.ascontiguousarray(v).astype(npdt) for k, v in w.items()}
    iota128 = np.ascontiguousarray(
        np.tile(np.array([0, 1, 2, 3, 4] * 3 + [0], np.float32), 8)
    ).reshape(128, 1)

    in_maps = []
    for c in range(NCORES):
        s = slice(c * RPC, (c + 1) * RPC)
        m = {"curT": np.ascontiguousarray(curT[:, s]),
             "nxtT": np.ascontiguousarray(nxtT[:, s]),
             "act128": np.ascontiguousarray(act128[:, s]),
             "iota128": iota128}
        m.update(w)
        in_maps.append(m)
    return in_maps


def _assemble_outputs(results):
    predT = np.concatenate([np.asarray(r["predT"], np.float32)
                            for r in results], axis=1)          # [128, ROWS]
    logitsT = np.concatenate([np.asarray(r["logitsT"], np.float32)
                              for r in results], axis=1)        # [21, ROWS]
    pred = predT.T.reshape(F_FRAMES, A_AGENTS, FEAT).astype(np.float32)
    lg = logitsT.T                                              # [ROWS, 21]
    logits0 = lg[:, 0:5].reshape(F_FRAMES, A_AGENTS, 5).astype(np.float32)
    logits1 = lg[:, 5:12].reshape(F_FRAMES, A_AGENTS, 7).astype(np.float32)
    logits2 = lg[:, 12:21].reshape(F_FRAMES, A_AGENTS, 9).astype(np.float32)
    return pred, logits0, logits1, logits2


def kernel(**inputs):
    variant = VARIANT
    nc = _get_program(variant)
    in_maps = _prep_inputs(inputs, variant)
    res = run_bass_kernel_spmd(nc, in_maps, list(range(NCORES)))
    return _assemble_outputs(res.results)


# revision 9
# speedup vs baseline: 3.0543x; 1.1537x over previous
"""Trainium2 Bass kernel for nn_AdversarialHead (scatter_memory).

Computes, for F*A = 131072 rows:
  one_hot  = multi-hot(actions + starts)                      [rows, 21]
  h        = leaky_relu(fc1_w @ [cur; one_hot] + fc1_b, 0.1)  [rows, 140]
  pred     = fc2_w @ h + fc2_b                                [rows, 128]
  logits_i = head_wi @ [cur; nxt] + head_bi                   [rows, 5/7/9]

Strategy (pure data parallel over 8 cores, 16384 rows each):
- Host pre-transposes cur/nxt to [feat, rows] so every device matmul is a
  plain orientation-B matmul (features on partitions, rows on the free dim).
- The one-hot scatter becomes: per-partition is_equal against an iota
  constant (building one-hotT tiles on chip), then a small-K matmul against
  host-prebuilt embedding tables (fc1_w columns re-indexed by action value).
  All biases are folded into the row-15 "ones" row of those tables.
- leaky_relu is decomposed as 0.1*x + 0.9*relu(x); the linear 0.1*x path is
  folded into the weights (Wfuse = 0.1*fc2_w@fc1_w) so the device only needs
  a single Relu per PSUM tile; the relu path uses 0.9-scaled fc2 weights.
- f32 mode uses float32r matmuls (full-rate PE); bf16 mode halves HBM bytes.

Outputs are produced transposed ([feat, rows]) and un-transposed on host.
"""
import os

import numpy as np
import ml_dtypes

import concourse.bass as bass
import concourse.tile as tile
from concourse import bacc, mybir
from concourse.bass_utils import run_bass_kernel_spmd

# ---------------------------------------------------------------- constants
F_FRAMES, A_AGENTS, FEAT = 4096, 32, 128
NVEC = [5, 7, 9]
STARTS = [0, 5, 12]
ACT_TOTAL = 21            # sum(NVEC)
HID = 140
NCORES = 8
ROWS = F_FRAMES * A_AGENTS          # 131072
RPC = ROWS // NCORES                # 16384 rows per core
TILE_N = 1024                       # rows per megatile
NT = RPC // TILE_N                  # 16 megatiles per core
HID2 = HID - FEAT                   # 12 hidden dims beyond partition 128

# psum2 layout: rows 0:21 = logits, rows 21:32 = zero pad, rows 32:44 = h2
# (compute-engine PSUM reads must start at a 32-aligned partition)
NL = 44
H2_BASE = 32

# Variant: "f32" (float32 I/O, float32r matmuls) or "bf16"
VARIANT = os.environ.get("KERNEL_DTYPE", "f32")

_prog_cache = {}


def _np_dt(variant):
    return np.float32 if variant == "f32" else ml_dtypes.bfloat16


def _prep_weights(fc1_w, fc1_b, fc2_w, fc2_b,
                  head_w0, head_b0, head_w1, head_b1, head_w2, head_b2):
    """Build all stationary (lhsT) operands in float32. Shapes noted as [K, M]."""
    f32 = np.float32
    fc1_w = np.asarray(fc1_w, f32); fc1_b = np.asarray(fc1_b, f32)
    fc2_w = np.asarray(fc2_w, f32); fc2_b = np.asarray(fc2_b, f32)
    wcat = np.concatenate([np.asarray(head_w0, f32), np.asarray(head_w1, f32),
                           np.asarray(head_w2, f32)], axis=0)          # [21, 256]
    bcat = np.concatenate([np.asarray(head_b0, f32), np.asarray(head_b1, f32),
                           np.asarray(head_b2, f32)], axis=0)          # [21]

    wfuse = 0.1 * (fc2_w @ fc1_w)                                      # [128, 149]

    # embedding row index map: p = t*5 + v  ->  fc1 input column 128+STARTS[t]+v
    emb_cols = [FEAT + STARTS[t] + v for t in range(3) for v in range(5)]

    # Every lhsT is zero-padded to [128, 128]: uniform full-array matmuls
    # keep the PE pipeline dense (small-K / partial-M matmuls measured 2.8x
    # slower and prevent the clock gate from warming). Zero weight rows make
    # the padded contraction exact as long as the rhs rows are finite.
    w = {k: np.zeros((FEAT, FEAT), f32) for k in
         ("wa", "wb", "wc", "wd", "we", "wi", "wj", "wf2", "wg2")}
    w["wa"][:, :] = fc1_w[:FEAT, :FEAT].T               # h1 from cur
    w["wb"][:, :ACT_TOTAL] = wcat[:, :FEAT].T           # logits+h2 from cur
    w["wb"][:, H2_BASE:H2_BASE + HID2] = fc1_w[FEAT:, :FEAT].T
    w["wc"][:15, :] = fc1_w[:FEAT, emb_cols].T          # h1 from one-hot
    w["wc"][15, :] = fc1_b[:FEAT]
    w["wd"][:15, H2_BASE:H2_BASE + HID2] = fc1_w[FEAT:, emb_cols].T
    w["wd"][15, H2_BASE:H2_BASE + HID2] = fc1_b[FEAT:]
    w["wd"][15, :ACT_TOTAL] = bcat                      # h2 + all biases
    w["we"][:, :ACT_TOTAL] = wcat[:, FEAT:].T           # logits from nxt
    w["wi"][:, :] = wfuse[:, :FEAT].T                   # 0.1-path from cur
    w["wj"][:15, :] = wfuse[:, emb_cols].T              # 0.1-path from one-hot
    w["wj"][15, :] = 0.1 * (fc2_w @ fc1_b) + fc2_b
    w["wf2"][:, :] = (0.9 * fc2_w[:, :FEAT]).T          # relu path h1
    w["wg2"][H2_BASE:H2_BASE + HID2, :] = (0.9 * fc2_w[:, FEAT:]).T
    return w


def _build_program(variant):
    """Build the SPMD Bass program (identical across cores)."""
    if variant == "f32":
        dt_dat = mybir.dt.float32r     # matmul operands, stored as f32
        dt_out = mybir.dt.float32
    else:
        dt_dat = mybir.dt.bfloat16
        dt_out = mybir.dt.bfloat16
    dt_act = mybir.dt.bfloat16         # action values (0..4, exact in bf16)
    f32 = mybir.dt.float32

    AF = mybir.ActivationFunctionType
    OP = mybir.AluOpType

    nc = bacc.Bacc(None, target_bir_lowering=False, debug=False)

    def din(name, shape, dt):
        return nc.dram_tensor(name, list(shape), dt, kind="ExternalInput").ap()

    def dout(name, shape, dt):
        return nc.dram_tensor(name, list(shape), dt, kind="ExternalOutput").ap()

    curT = din("curT", (FEAT, RPC), dt_dat)
    nxtT = din("nxtT", (FEAT, RPC), dt_dat)
    act16 = din("act16", (16, RPC), dt_act)
    w_names = ("wa", "wb", "wc", "wd", "we", "wi", "wj", "wf2", "wg2")
    w_dram = {k: din(k, (FEAT, FEAT), dt_dat) for k in w_names}
    iota128 = din("iota128", (128, 1), f32)
    predT = dout("predT", (FEAT, RPC), dt_out)
    logitsT = dout("logitsT", (ACT_TOTAL, RPC), f32)

    with tile.TileContext(nc) as tc:
        with (
            tc.tile_pool(name="consts", bufs=1) as cp,
            tc.tile_pool(name="io", bufs=3) as iop,
            tc.tile_pool(name="mid", bufs=3) as mp,
            tc.tile_pool(name="pp1", bufs=2, space="PSUM") as pp1,
            tc.tile_pool(name="pp2", bufs=1, space="PSUM") as pp2,
            tc.tile_pool(name="pp3", bufs=1, space="PSUM") as pp3,
        ):
            # ---- load constants once ----
            cw = {}
            for k in w_names:
                t = cp.tile([FEAT, FEAT], dt_dat, name=f"c_{k}")
                nc.sync.dma_start(t[:], w_dram[k])
                cw[k] = t
            iota_t = cp.tile([128, 1], f32, name="c_iota")
            nc.sync.dma_start(iota_t[:], iota128)

            for it in range(NT):
                sl = slice(it * TILE_N, (it + 1) * TILE_N)
                cur_t = iop.tile([FEAT, TILE_N], dt_dat, tag="cur")
                nc.sync.dma_start(cur_t[:], curT[:, sl])
                nxt_t = iop.tile([FEAT, TILE_N], dt_dat, tag="nxt")
                nc.sync.dma_start(nxt_t[:], nxtT[:, sl])
                act_t = iop.tile([128, TILE_N], dt_act, tag="act")
                nc.sync.dma_start(act_t[0:16, :], act16[:, sl])

                # one-hotT tile; only rows 0:16 are loaded — rows 16:127
                # compare stale SBUF data, but is_equal always yields finite
                # 0/1 and those rows hit zero-padded weight rows, so the
                # matmul contribution is exactly zero.
                oh = mp.tile([128, TILE_N], dt_dat, tag="oh")
                nc.vector.tensor_scalar(oh[:], act_t[:], iota_t[:], None,
                                        OP.is_equal)

                p1 = pp1.tile([FEAT, TILE_N], f32, tag="p1")   # h1 pre-act
                p2 = pp2.tile([FEAT, TILE_N], f32, tag="p2")   # logits + h2
                p3 = pp3.tile([FEAT, TILE_N], f32, tag="p3")   # pred accum

                # each 512-wide PSUM bank region is its own accumulation
                # group: first writer sets start=True, last sets stop=True.
                # MMs are grouped by PSUM bank — consecutive bank switches
                # micro-idle the PE and keep the HAM clock gate cold.
                for s in range(TILE_N // 512):
                    hs = slice(s * 512, (s + 1) * 512)
                    nc.tensor.matmul(p1[:, hs], cw["wa"][:], cur_t[:, hs],
                                     start=True, stop=False)
                    nc.tensor.matmul(p1[:, hs], cw["wc"][:], oh[:, hs],
                                     start=False, stop=True)
                for s in range(TILE_N // 512):
                    hs = slice(s * 512, (s + 1) * 512)
                    nc.tensor.matmul(p2[:, hs], cw["wb"][:], cur_t[:, hs],
                                     start=True, stop=False)
                    nc.tensor.matmul(p2[:, hs], cw["wd"][:], oh[:, hs],
                                     start=False, stop=False)
                    nc.tensor.matmul(p2[:, hs], cw["we"][:], nxt_t[:, hs],
                                     start=False, stop=True)
                for s in range(TILE_N // 512):
                    hs = slice(s * 512, (s + 1) * 512)
                    nc.tensor.matmul(p3[:, hs], cw["wi"][:], cur_t[:, hs],
                                     start=True, stop=False)
                    nc.tensor.matmul(p3[:, hs], cw["wj"][:], oh[:, hs],
                                     start=False, stop=False)

                h1 = mp.tile([FEAT, TILE_N], dt_dat, tag="h1")
                nc.scalar.activation(h1[:], p1[:], AF.Relu)
                hl2 = mp.tile([FEAT, TILE_N], dt_dat, tag="hl2")
                nc.scalar.activation(hl2[:], p2[:], AF.Relu)
                lg = mp.tile([ACT_TOTAL, TILE_N], f32, tag="lg")
                nc.vector.tensor_copy(lg[:], p2[0:ACT_TOTAL, :])

                for s in range(TILE_N // 512):
                    hs = slice(s * 512, (s + 1) * 512)
                    nc.tensor.matmul(p3[:, hs], cw["wf2"][:], h1[:, hs],
                                     start=False, stop=False)
                    nc.tensor.matmul(p3[:, hs], cw["wg2"][:], hl2[:, hs],
                                     start=False, stop=True)

                pred = mp.tile([FEAT, TILE_N], dt_out, tag="pred")
                nc.vector.tensor_copy(pred[:], p3[:])
                nc.gpsimd.dma_start(predT[:, sl], pred[:])
                nc.gpsimd.dma_start(logitsT[:, sl], lg[:])

    nc.compile()
    return nc


def _get_program(variant):
    if variant not in _prog_cache:
        _prog_cache[variant] = _build_program(variant)
    return _prog_cache[variant]


def _prep_inputs(inputs, variant):
    """Host-side data staging: transpose/shard/cast. Returns per-core in_maps."""
    npdt = _np_dt(variant)
    cur = np.asarray(inputs["current_feature"], np.float32).reshape(ROWS, FEAT)
    nxt = np.asarray(inputs["next_feature"], np.float32).reshape(ROWS, FEAT)
    acts = np.asarray(inputs["actions"])                       # [F, 3, A] int32

    curT = np.ascontiguousarray(cur.T).astype(npdt)            # [128, ROWS]
    nxtT = np.ascontiguousarray(nxt.T).astype(npdt)
    a3 = np.transpose(acts, (1, 0, 2)).reshape(3, ROWS)        # row t = type t
    act16 = np.zeros((16, ROWS), ml_dtypes.bfloat16)
    act16[:15] = np.repeat(a3, 5, axis=0).astype(ml_dtypes.bfloat16)

    w = _prep_weights(
        inputs["fc1_w"], inputs["fc1_b"], inputs["fc2_w"], inputs["fc2_b"],
        inputs["head_w0"], inputs["head_b0"], inputs["head_w1"],
        inputs["head_b1"], inputs["head_w2"], inputs["head_b2"])
    w = {k: np.ascontiguousarray(v).astype(npdt) for k, v in w.items()}
    iota128 = np.ascontiguousarray(
        np.tile(np.array([0, 1, 2, 3, 4] * 3 + [0], np.float32), 8)
    ).reshape(128, 1)

    in_maps = []
    for c in range(NCORES):
        s = slice(c * RPC, (c + 1) * RPC)
        m = {"curT": np.ascontiguousarray(curT[:, s]),
             "nxtT": np.ascontiguousarray(nxtT[:, s]),
             "act16": np.ascontiguousarray(act16[:, s]),
             "iota128": iota128}
        m.update(w)
        in_maps.append(m)
    return in_maps


def _assemble_outputs(results):
    predT = np.concatenate([np.asarray(r["predT"], np.float32)
                            for r in results], axis=1)          # [128, ROWS]
    logitsT = np.concatenate([np.asarray(r["logitsT"], np.float32)
                              for r in results], axis=1)        # [21, ROWS]
    pred = predT.T.reshape(F_FRAMES, A_AGENTS, FEAT).astype(np.float32)
    lg = logitsT.T                                              # [ROWS, 21]
    logits0 = lg[:, 0:5].reshape(F_FRAMES, A_AGENTS, 5).astype(np.float32)
    logits1 = lg[:, 5:12].reshape(F_FRAMES, A_AGENTS, 7).astype(np.float32)
    logits2 = lg[:, 12:21].reshape(F_FRAMES, A_AGENTS, 9).astype(np.float32)
    return pred, logits0, logits1, logits2


def kernel(**inputs):
    variant = VARIANT
    nc = _get_program(variant)
    in_maps = _prep_inputs(inputs, variant)
    res = run_bass_kernel_spmd(nc, in_maps, list(range(NCORES)))
    return _assemble_outputs(res.results)


# revision 10
# speedup vs baseline: 3.0694x; 1.0050x over previous
"""Trainium2 Bass kernel for nn_AdversarialHead (scatter_memory).

Computes, for F*A = 131072 rows:
  one_hot  = multi-hot(actions + starts)                      [rows, 21]
  h        = leaky_relu(fc1_w @ [cur; one_hot] + fc1_b, 0.1)  [rows, 140]
  pred     = fc2_w @ h + fc2_b                                [rows, 128]
  logits_i = head_wi @ [cur; nxt] + head_bi                   [rows, 5/7/9]

Strategy (pure data parallel over 8 cores, 16384 rows each):
- Host pre-transposes cur/nxt to [feat, rows] so every device matmul is a
  plain orientation-B matmul (features on partitions, rows on the free dim).
- The one-hot scatter becomes: per-partition is_equal against an iota
  constant (building one-hotT tiles on chip), then a small-K matmul against
  host-prebuilt embedding tables (fc1_w columns re-indexed by action value).
  All biases are folded into the row-15 "ones" row of those tables.
- leaky_relu is decomposed as 0.1*x + 0.9*relu(x); the linear 0.1*x path is
  folded into the weights (Wfuse = 0.1*fc2_w@fc1_w) so the device only needs
  a single Relu per PSUM tile; the relu path uses 0.9-scaled fc2 weights.
- f32 mode uses float32r matmuls (full-rate PE); bf16 mode halves HBM bytes.

Outputs are produced transposed ([feat, rows]) and un-transposed on host.
"""
import os

import numpy as np
import ml_dtypes

import concourse.bass as bass
import concourse.tile as tile
from concourse import bacc, mybir
from concourse.bass_utils import run_bass_kernel_spmd

# ---------------------------------------------------------------- constants
F_FRAMES, A_AGENTS, FEAT = 4096, 32, 128
NVEC = [5, 7, 9]
STARTS = [0, 5, 12]
ACT_TOTAL = 21            # sum(NVEC)
HID = 140
NCORES = 8
ROWS = F_FRAMES * A_AGENTS          # 131072
RPC = ROWS // NCORES                # 16384 rows per core
TILE_N = 1024                       # rows per megatile
NT = RPC // TILE_N                  # 16 megatiles per core
HID2 = HID - FEAT                   # 12 hidden dims beyond partition 128

# psum2 layout: rows 0:21 = logits, rows 21:32 = zero pad, rows 32:44 = h2
# (compute-engine PSUM reads must start at a 32-aligned partition)
NL = 44
H2_BASE = 32

# Variant: "f32" (float32 I/O, float32r matmuls) or "bf16"
VARIANT = os.environ.get("KERNEL_DTYPE", "f32")

_prog_cache = {}


def _np_dt(variant):
    return np.float32 if variant == "f32" else ml_dtypes.bfloat16


def _prep_weights(fc1_w, fc1_b, fc2_w, fc2_b,
                  head_w0, head_b0, head_w1, head_b1, head_w2, head_b2):
    """Build all stationary (lhsT) operands in float32. Shapes noted as [K, M]."""
    f32 = np.float32
    fc1_w = np.asarray(fc1_w, f32); fc1_b = np.asarray(fc1_b, f32)
    fc2_w = np.asarray(fc2_w, f32); fc2_b = np.asarray(fc2_b, f32)
    wcat = np.concatenate([np.asarray(head_w0, f32), np.asarray(head_w1, f32),
                           np.asarray(head_w2, f32)], axis=0)          # [21, 256]
    bcat = np.concatenate([np.asarray(head_b0, f32), np.asarray(head_b1, f32),
                           np.asarray(head_b2, f32)], axis=0)          # [21]

    wfuse = 0.1 * (fc2_w @ fc1_w)                                      # [128, 149]

    # embedding row index map: p = t*5 + v  ->  fc1 input column 128+STARTS[t]+v
    emb_cols = [FEAT + STARTS[t] + v for t in range(3) for v in range(5)]

    # Every lhsT is zero-padded to [128, 128]: uniform full-array matmuls
    # keep the PE pipeline dense (small-K / partial-M matmuls measured 2.8x
    # slower and prevent the clock gate from warming). Zero weight rows make
    # the padded contraction exact as long as the rhs rows are finite.
    w = {k: np.zeros((FEAT, FEAT), f32) for k in
         ("wa", "wb", "wc", "wd", "we", "wi", "wj", "wf2", "wg2")}
    w["wa"][:, :] = fc1_w[:FEAT, :FEAT].T               # h1 from cur
    w["wb"][:, :ACT_TOTAL] = wcat[:, :FEAT].T           # logits+h2 from cur
    w["wb"][:, H2_BASE:H2_BASE + HID2] = fc1_w[FEAT:, :FEAT].T
    w["wc"][:15, :] = fc1_w[:FEAT, emb_cols].T          # h1 from one-hot
    w["wc"][15, :] = fc1_b[:FEAT]
    w["wd"][:15, H2_BASE:H2_BASE + HID2] = fc1_w[FEAT:, emb_cols].T
    w["wd"][15, H2_BASE:H2_BASE + HID2] = fc1_b[FEAT:]
    w["wd"][15, :ACT_TOTAL] = bcat                      # h2 + all biases
    w["we"][:, :ACT_TOTAL] = wcat[:, FEAT:].T           # logits from nxt
    w["wi"][:, :] = wfuse[:, :FEAT].T                   # 0.1-path from cur
    w["wj"][:15, :] = wfuse[:, emb_cols].T              # 0.1-path from one-hot
    w["wj"][15, :] = 0.1 * (fc2_w @ fc1_b) + fc2_b
    w["wf2"][:, :] = (0.9 * fc2_w[:, :FEAT]).T          # relu path h1
    w["wg2"][H2_BASE:H2_BASE + HID2, :] = (0.9 * fc2_w[:, FEAT:]).T
    return w


def _build_program(variant):
    """Build the SPMD Bass program (identical across cores)."""
    if variant == "f32":
        dt_dat = mybir.dt.float32r     # matmul operands, stored as f32
        dt_out = mybir.dt.float32
    else:
        dt_dat = mybir.dt.bfloat16
        dt_out = mybir.dt.bfloat16
    dt_act = mybir.dt.bfloat16         # action values (0..4, exact in bf16)
    f32 = mybir.dt.float32

    AF = mybir.ActivationFunctionType
    OP = mybir.AluOpType

    nc = bacc.Bacc(None, target_bir_lowering=False, debug=False)

    def din(name, shape, dt):
        return nc.dram_tensor(name, list(shape), dt, kind="ExternalInput").ap()

    def dout(name, shape, dt):
        return nc.dram_tensor(name, list(shape), dt, kind="ExternalOutput").ap()

    curT = din("curT", (FEAT, RPC), dt_dat)
    nxtT = din("nxtT", (FEAT, RPC), dt_dat)
    act16 = din("act16", (16, RPC), dt_act)
    w_names = ("wa", "wb", "wc", "wd", "we", "wi", "wj", "wf2", "wg2")
    w_dram = {k: din(k, (FEAT, FEAT), dt_dat) for k in w_names}
    iota128 = din("iota128", (128, 1), f32)
    predT = dout("predT", (FEAT, RPC), dt_out)
    logitsT = dout("logitsT", (ACT_TOTAL, RPC), f32)

    with tile.TileContext(nc) as tc:
        with (
            tc.tile_pool(name="consts", bufs=1) as cp,
            tc.tile_pool(name="io", bufs=3) as iop,
            tc.tile_pool(name="mid", bufs=3) as mp,
            tc.tile_pool(name="pp1", bufs=2, space="PSUM") as pp1,
            tc.tile_pool(name="pp2", bufs=1, space="PSUM") as pp2,
            tc.tile_pool(name="pp3", bufs=1, space="PSUM") as pp3,
        ):
            # ---- load constants once ----
            cw = {}
            for k in w_names:
                t = cp.tile([FEAT, FEAT], dt_dat, name=f"c_{k}")
                nc.sync.dma_start(t[:], w_dram[k])
                cw[k] = t
            iota_t = cp.tile([128, 1], f32, name="c_iota")
            nc.sync.dma_start(iota_t[:], iota128)

            for it in range(NT):
                sl = slice(it * TILE_N, (it + 1) * TILE_N)
                # big DMAs are split in partition halves so the two chunks
                # land on different HW queues (halves the transfer latency)
                cur_t = iop.tile([FEAT, TILE_N], dt_dat, tag="cur")
                nc.sync.dma_start(cur_t[0:64, :], curT[0:64, sl])
                nc.sync.dma_start(cur_t[64:128, :], curT[64:128, sl])
                nxt_t = iop.tile([FEAT, TILE_N], dt_dat, tag="nxt")
                nc.sync.dma_start(nxt_t[0:64, :], nxtT[0:64, sl])
                nc.sync.dma_start(nxt_t[64:128, :], nxtT[64:128, sl])
                act_t = iop.tile([128, TILE_N], dt_act, tag="act")
                nc.sync.dma_start(act_t[0:16, :], act16[:, sl])

                # one-hotT tile; only rows 0:16 are loaded — rows 16:127
                # compare stale SBUF data, but is_equal always yields finite
                # 0/1 and those rows hit zero-padded weight rows, so the
                # matmul contribution is exactly zero.
                oh = mp.tile([128, TILE_N], dt_dat, tag="oh")
                nc.vector.tensor_scalar(oh[:], act_t[:], iota_t[:], None,
                                        OP.is_equal)

                p1 = pp1.tile([FEAT, TILE_N], f32, tag="p1")   # h1 pre-act
                p2 = pp2.tile([FEAT, TILE_N], f32, tag="p2")   # logits + h2
                p3 = pp3.tile([FEAT, TILE_N], f32, tag="p3")   # pred accum

                # each 512-wide PSUM bank region is its own accumulation
                # group: first writer sets start=True, last sets stop=True.
                # MMs are grouped by PSUM bank — consecutive bank switches
                # micro-idle the PE and keep the HAM clock gate cold.
                for s in range(TILE_N // 512):
                    hs = slice(s * 512, (s + 1) * 512)
                    nc.tensor.matmul(p1[:, hs], cw["wa"][:], cur_t[:, hs],
                                     start=True, stop=False)
                    nc.tensor.matmul(p1[:, hs], cw["wc"][:], oh[:, hs],
                                     start=False, stop=True)
                for s in range(TILE_N // 512):
                    hs = slice(s * 512, (s + 1) * 512)
                    nc.tensor.matmul(p2[:, hs], cw["wb"][:], cur_t[:, hs],
                                     start=True, stop=False)
                    nc.tensor.matmul(p2[:, hs], cw["wd"][:], oh[:, hs],
                                     start=False, stop=False)
                    nc.tensor.matmul(p2[:, hs], cw["we"][:], nxt_t[:, hs],
                                     start=False, stop=True)
                for s in range(TILE_N // 512):
                    hs = slice(s * 512, (s + 1) * 512)
                    nc.tensor.matmul(p3[:, hs], cw["wi"][:], cur_t[:, hs],
                                     start=True, stop=False)
                    nc.tensor.matmul(p3[:, hs], cw["wj"][:], oh[:, hs],
                                     start=False, stop=False)

                h1 = mp.tile([FEAT, TILE_N], dt_dat, tag="h1")
                nc.scalar.activation(h1[:], p1[:], AF.Relu)
                hl2 = mp.tile([FEAT, TILE_N], dt_dat, tag="hl2")
                nc.scalar.activation(hl2[:], p2[:], AF.Relu)
                lg = mp.tile([ACT_TOTAL, TILE_N], f32, tag="lg")
                nc.vector.tensor_copy(lg[:], p2[0:ACT_TOTAL, :])

                for s in range(TILE_N // 512):
                    hs = slice(s * 512, (s + 1) * 512)
                    nc.tensor.matmul(p3[:, hs], cw["wf2"][:], h1[:, hs],
                                     start=False, stop=False)
                    nc.tensor.matmul(p3[:, hs], cw["wg2"][:], hl2[:, hs],
                                     start=False, stop=True)

                pred = mp.tile([FEAT, TILE_N], dt_out, tag="pred")
                nc.vector.tensor_copy(pred[:], p3[:])
                nc.gpsimd.dma_start(predT[0:64, sl], pred[0:64, :])
                nc.gpsimd.dma_start(predT[64:128, sl], pred[64:128, :])
                nc.gpsimd.dma_start(logitsT[:, sl], lg[:])

    nc.compile()
    return nc


def _get_program(variant):
    if variant not in _prog_cache:
        _prog_cache[variant] = _build_program(variant)
    return _prog_cache[variant]


def _prep_inputs(inputs, variant):
    """Host-side data staging: transpose/shard/cast. Returns per-core in_maps."""
    npdt = _np_dt(variant)
    cur = np.asarray(inputs["current_feature"], np.float32).reshape(ROWS, FEAT)
    nxt = np.asarray(inputs["next_feature"], np.float32).reshape(ROWS, FEAT)
    acts = np.asarray(inputs["actions"])                       # [F, 3, A] int32

    curT = np.ascontiguousarray(cur.T).astype(npdt)            # [128, ROWS]
    nxtT = np.ascontiguousarray(nxt.T).astype(npdt)
    a3 = np.transpose(acts, (1, 0, 2)).reshape(3, ROWS)        # row t = type t
    act16 = np.zeros((16, ROWS), ml_dtypes.bfloat16)
    act16[:15] = np.repeat(a3, 5, axis=0).astype(ml_dtypes.bfloat16)

    w = _prep_weights(
        inputs["fc1_w"], inputs["fc1_b"], inputs["fc2_w"], inputs["fc2_b"],
        inputs["head_w0"], inputs["head_b0"], inputs["head_w1"],
        inputs["head_b1"], inputs["head_w2"], inputs["head_b2"])
    w = {k: np.ascontiguousarray(v).astype(npdt) for k, v in w.items()}
    iota128 = np.ascontiguousarray(
        np.tile(np.array([0, 1, 2, 3, 4] * 3 + [0], np.float32), 8)
    ).reshape(128, 1)

    in_maps = []
    for c in range(NCORES):
        s = slice(c * RPC, (c + 1) * RPC)
        m = {"curT": np.ascontiguousarray(curT[:, s]),
             "nxtT": np.ascontiguousarray(nxtT[:, s]),
             "act16": np.ascontiguousarray(act16[:, s]),
             "iota128": iota128}
        m.update(w)
        in_maps.append(m)
    return in_maps


def _assemble_outputs(results):
    predT = np.concatenate([np.asarray(r["predT"], np.float32)
                            for r in results], axis=1)          # [128, ROWS]
    logitsT = np.concatenate([np.asarray(r["logitsT"], np.float32)
                              for r in results], axis=1)        # [21, ROWS]
    pred = predT.T.reshape(F_FRAMES, A_AGENTS, FEAT).astype(np.float32)
    lg = logitsT.T                                              # [ROWS, 21]
    logits0 = lg[:, 0:5].reshape(F_FRAMES, A_AGENTS, 5).astype(np.float32)
    logits1 = lg[:, 5:12].reshape(F_FRAMES, A_AGENTS, 7).astype(np.float32)
    logits2 = lg[:, 12:21].reshape(F_FRAMES, A_AGENTS, 9).astype(np.float32)
    return pred, logits0, logits1, logits2


def kernel(**inputs):
    variant = VARIANT
    nc = _get_program(variant)
    in_maps = _prep_inputs(inputs, variant)
    res = run_bass_kernel_spmd(nc, in_maps, list(range(NCORES)))
    return _assemble_outputs(res.results)
